# revision 43
# baseline (speedup 1.0000x reference)
"""BinaryGPTNeoBlock on 8 trn2 NeuronCores.

Sequence-parallel over 8 cores: core c owns rows {c, c+8, ...} of both
batches (256 per batch, 512 total); causality is per-core mask data so
the program stays SPMD-uniform. K/V are projected feature-/token-major
directly (no PE transposes), cast bf16, and AllGathered in two halves
each (interleaved with the projection passes) so attention starts with
no stall. MLP weights: each core tanh's + scales (x64) its 1/8th into
fp8, two AllGathers share them, and fc/proj run fp8 DoubleRow matmuls
(2x PE rate); the 1/64 descale folds into PSUM evacuation.

Self-contained: hardcodes shapes; host only shards/transposes/builds masks.
"""

import numpy as np
import ml_dtypes

import concourse.bass as bass
import concourse.tile as tile
from concourse import bacc, mybir
from concourse.bass_utils import run_bass_kernel_spmd
from concourse.masks import make_identity

B, S, D = 2, 2048, 2048
H = 16
HD = 128
FF = 4 * D
EPS = 1e-5
NC = 8
CH = 256               # q-chunk length (S // NC)
TL = 2 * CH            # 512 local rows (one chunk per batch)
WFC_CH = 256 * FF      # own d-rows of wfcT
WPJ_CH = 1024 * D      # own f-rows of wpjT
WS = 64.0              # fp8 weight pre-scale (undone at PSUM evacuation)

F8_MLP = True          # fc/proj in fp8 DoubleRow
F8_OP = False          # out-proj in fp8 DoubleRow

dt = mybir.dt
AF = mybir.ActivationFunctionType
OP = mybir.AluOpType
DR = mybir.MatmulPerfMode.DoubleRow

_CACHE = {}


def _build(apply_g1, apply_b1, apply_g2, apply_b2):
    nc = bacc.Bacc("TRN2", target_bir_lowering=False, debug=False,
                   num_devices=NC)

    xl_d = nc.dram_tensor("xl", [TL, D], dt.float32, kind="ExternalInput").ap()
    wqT_d = nc.dram_tensor("wqT", [D, D], dt.float32, kind="ExternalInput").ap()
    wkT_d = nc.dram_tensor("wkT", [D, D], dt.float32, kind="ExternalInput").ap()
    wvT_d = nc.dram_tensor("wvT", [D, D], dt.float32, kind="ExternalInput").ap()
    woT_d = nc.dram_tensor("woT", [D, D], dt.float32, kind="ExternalInput").ap()
    wfc_ch_d = nc.dram_tensor("wfc_ch", [WFC_CH], dt.float32,
                              kind="ExternalInput").ap()
    wpj_ch_d = nc.dram_tensor("wpj_ch", [WPJ_CH], dt.float32,
                              kind="ExternalInput").ap()
    mask_d = nc.dram_tensor("mask", [128, 8, 384], dt.bfloat16,
                            kind="ExternalInput").ap()
    ln1g_d = nc.dram_tensor("ln1g", [D], dt.float32, kind="ExternalInput").ap()
    ln1b_d = nc.dram_tensor("ln1b", [D], dt.float32, kind="ExternalInput").ap()
    ln2g_d = nc.dram_tensor("ln2g", [D], dt.float32, kind="ExternalInput").ap()
    ln2b_d = nc.dram_tensor("ln2b", [D], dt.float32, kind="ExternalInput").ap()
    bo_d = nc.dram_tensor("bo", [D], dt.float32, kind="ExternalInput").ap()
    bfc_d = nc.dram_tensor("bfc", [FF], dt.float32, kind="ExternalInput").ap()
    bpj_d = nc.dram_tensor("bpj", [D], dt.float32, kind="ExternalInput").ap()
    out_d = nc.dram_tensor("out", [TL, D], dt.float32,
                           kind="ExternalOutput").ap()

    mlp_dt = dt.float8e4 if F8_MLP else dt.bfloat16
    op_dt = dt.float8e4 if F8_OP else dt.bfloat16
    KHALF = 1024 * TL        # elems of one kT half per core
    VHALF = TL * 1024

    def bcast_row(src_ap, n):
        return bass.AP(tensor=src_ap.tensor, offset=src_ap.offset,
                       ap=[[0, 128], [1, n]])

    with tile.TileContext(nc) as tc:
        import contextlib
        stack = contextlib.ExitStack()
        main = stack.enter_context(tc.tile_pool(name="main", bufs=1))
        dram = stack.enter_context(
            tc.tile_pool(name="dram", bufs=1, space="DRAM"))

        ident = main.tile([128, 128], dt.float32)
        make_identity(nc, ident[:])
        ident_b = main.tile([128, 128], dt.bfloat16)
        nc.vector.tensor_copy(ident_b[:], ident[:])
        ones_col_b = main.tile([128, 1], dt.bfloat16)
        nc.vector.memset(ones_col_b[:], 1.0)
        ones_row = main.tile([1, 128], dt.float32)
        nc.vector.memset(ones_row[:], 1.0)
        eps_t = main.tile([128, 1], dt.float32)
        nc.vector.memset(eps_t[:], EPS)
        bo_bc = main.tile([128, D], dt.float32)
        nc.sync.dma_start(out=bo_bc[:], in_=bcast_row(bo_d, D))
        bpj_bc = main.tile([128, D], dt.float32)
        nc.sync.dma_start(out=bpj_bc[:], in_=bcast_row(bpj_d, D))
        masks = main.tile([128, 8, 384], dt.bfloat16)
        nc.sync.dma_start(out=masks[:], in_=mask_d[:])
        bfc_pp = main.tile([128, FF // 128], dt.float32)
        nc.sync.dma_start(
            out=bfc_pp[:],
            in_=bass.AP(tensor=bfc_d.tensor, offset=bfc_d.offset,
                        ap=[[1, 128], [128, FF // 128]]))
        ln_bc = {}
        for nm, flag, src in (("g1", apply_g1, ln1g_d),
                              ("b1", apply_b1, ln1b_d),
                              ("g2", apply_g2, ln2g_d),
                              ("b2", apply_b2, ln2b_d)):
            if flag:
                t = main.tile([128, D], dt.float32, name=f"ln_{nm}")
                nc.sync.dma_start(out=t[:], in_=bcast_row(src, D))
                ln_bc[nm] = t

        # rotating big activation slots (16KB/part each, 2 slots)
        hT = main.tile([128, 16, 512], dt.bfloat16, tag="bigA", bufs=2,
                       name="hT")
        QT = main.tile([128, 16, 512], dt.bfloat16, tag="bigA", bufs=2,
                       name="QT")

        wfc_bounce = dram.tile([WFC_CH], mlp_dt)
        wpj_bounce = dram.tile([WPJ_CH], mlp_dt)
        wfc_gath = dram.tile([NC * WFC_CH], mlp_dt, addr_space="Shared")
        wpj_gath = dram.tile([NC * WPJ_CH], mlp_dt, addr_space="Shared")

        def wprep_emit():
            # tanh + x64 + fp8-cast of own 1/8th of the MLP weights
            with tc.tile_pool(name="wprep", bufs=3) as wprep:
                for src, dst, nrb, ncols in ((wfc_ch_d, wfc_bounce, 2, FF),
                                             (wpj_ch_d, wpj_bounce, 8, D)):
                    for rb in range(nrb):
                        for ci in range(ncols // 2048):
                            off = rb * 128 * ncols + ci * 2048
                            raw = wprep.tile([128, 2048], dt.float32,
                                             tag="wraw")
                            nc.sync.dma_start(
                                out=raw[:],
                                in_=bass.AP(tensor=src.tensor,
                                            offset=src.offset + off,
                                            ap=[[ncols, 128], [1, 2048]]))
                            tnh = wprep.tile([128, 2048], dt.float32,
                                             tag="wtnh")
                            nc.scalar.activation(tnh[:], raw[:], AF.Tanh)
                            w8 = wprep.tile([128, 2048], mlp_dt, tag="w8")
                            if F8_MLP:
                                nc.vector.tensor_scalar(w8[:], tnh[:], WS,
                                                        None, op0=OP.mult)
                            else:
                                nc.vector.tensor_copy(w8[:], tnh[:])
                            nc.sync.dma_start(
                                out=bass.AP(tensor=dst.tensor,
                                            offset=dst.offset + off,
                                            ap=[[ncols, 128], [1, 2048]]),
                                in_=w8[:])

        # ---------- Phase A: x -> LN1 -> h^T ----------
        def layernorm(x_t, h_t, gk, bk):
            with tc.tile_pool(name="lnp", bufs=2) as lp:
                st = lp.tile([128, 4, 6], dt.float32, tag="st")
                xr = x_t[:].rearrange("p (n f) -> p n f", n=4)
                for sg in range(4):
                    nc.vector.bn_stats(out=st[:, sg, :], in_=xr[:, sg, :])
                mv = lp.tile([128, 2], dt.float32, tag="mv")
                nc.vector.bn_aggr(out=mv[:], in_=st[:])
                std = lp.tile([128, 1], dt.float32, tag="sd")
                nc.scalar.activation(std[:], mv[:, 1:2], AF.Sqrt,
                                     bias=eps_t[:])
                rstd = lp.tile([128, 1], dt.float32, tag="rs")
                nc.vector.reciprocal(rstd[:], std[:])
                nc.vector.tensor_scalar(h_t[:], x_t[:], mv[:, 0:1], rstd[:],
                                        op0=OP.subtract, op1=OP.mult)
                if gk in ln_bc:
                    nc.vector.tensor_mul(h_t[:], h_t[:], ln_bc[gk][:])
                if bk in ln_bc:
                    nc.vector.tensor_add(h_t[:], h_t[:], ln_bc[bk][:])

        with tc.tile_pool(name="xa", bufs=2) as xa, \
             tc.tile_pool(name="ha", bufs=1) as ha, \
             tc.tile_pool(name="trps", bufs=4, space="PSUM") as trps:
            h_ts = []
            for tb in range(4):
                x_t = xa.tile([128, D], dt.float32, tag="x")
                nc.sync.dma_start(out=x_t[:],
                                  in_=xl_d[tb * 128:(tb + 1) * 128, :])
                h_t = ha.tile([128, D], dt.float32, tag=f"h{tb}",
                              name=f"h_{tb}")
                layernorm(x_t, h_t, "g1", "b1")
                h_ts.append(h_t)
            # dj-major so hT[:, dj, :] completes early -> K matmuls overlap
            for dj in range(16):
                for tb in range(4):
                    ps = trps.tile([128, 128], dt.float32, tag="tp")
                    nc.tensor.transpose(
                        ps[:], h_ts[tb][:, dj * 128:(dj + 1) * 128],
                        ident[:])
                    nc.vector.tensor_copy(hT[:, dj, tb * 128:(tb + 1) * 128],
                                          ps[:])

        # ---------- Phase B: QKV (feature-major K/Q, token-major V) ----
        k_bounce = [dram.tile([KHALF], dt.bfloat16, name=f"kb{i}")
                    for i in range(2)]
        v_bounce = [dram.tile([VHALF], dt.bfloat16, name=f"vb{i}")
                    for i in range(2)]
        k_gath = [dram.tile([NC * KHALF], dt.bfloat16, addr_space="Shared",
                            name=f"kg{i}") for i in range(2)]
        v_gath = [dram.tile([NC * VHALF], dt.bfloat16, addr_space="Shared",
                            name=f"vg{i}") for i in range(2)]

        qkv_pool = tc.tile_pool(name="qkv", bufs=3)
        qkvp = qkv_pool.__enter__()
        qkv_ps_pool = tc.tile_pool(name="qkvps", bufs=1, space="PSUM")
        qkvps = qkv_ps_pool.__enter__()
        kacc_pool = tc.tile_pool(name="kacc", bufs=1)
        kaccp = kacc_pool.__enter__()
        kacc = kaccp.tile([128, 16, 512], dt.bfloat16, name="kacc")
        vacc = [kaccp.tile([128, D], dt.bfloat16, name=f"vacc{t}")
                for t in range(4)]

        def proj_fmajor(wT_dram, pss, dest):
            # features [pss*1024, pss*1024+1024) of w^T h^T -> dest[:, 8pss..]
            ps = [qkvps.tile([128, 512], dt.float32, tag=f"q{i}",
                             name=f"ps_{wT_dram.tensor.name}_{pss}_{i}")
                  for i in range(8)]
            for dj in range(16):
                raw = qkvp.tile([128, 1024], dt.float32, tag="qkraw",
                                bufs=10)
                nc.sync.dma_start(
                    out=raw[:],
                    in_=wT_dram[dj * 128:(dj + 1) * 128,
                                pss * 1024:(pss + 1) * 1024])
                wt = qkvp.tile([128, 1024], dt.bfloat16, tag="qktnh", bufs=4)
                nc.scalar.activation(wt[:], raw[:], AF.Tanh)
                for ft in range(8):
                    nc.tensor.matmul(ps[ft][:],
                                     wt[:, ft * 128:(ft + 1) * 128],
                                     hT[:, dj, :],
                                     start=(dj == 0), stop=(dj == 15))
            for ft in range(8):
                nc.vector.tensor_copy(dest[:, pss * 8 + ft, :], ps[ft][:])

        def proj_v(fgp):
            # token-major v for features [fgp*1024, fgp*1024+1024)
            ps = [qkvps.tile([128, 512], dt.float32, tag=f"q{i}",
                             name=f"ps_v_{fgp}_{i}")
                  for i in range(8)]
            for dj in range(16):
                raw = qkvp.tile([128, 1024], dt.float32, tag="qkraw",
                                bufs=10)
                nc.sync.dma_start(
                    out=raw[:],
                    in_=wvT_d[dj * 128:(dj + 1) * 128,
                              fgp * 1024:(fgp + 1) * 1024])
                wt = qkvp.tile([128, 1024], dt.bfloat16, tag="qktnh", bufs=4)
                nc.scalar.activation(wt[:], raw[:], AF.Tanh)
                for tt in range(4):
                    for fg2 in range(2):
                        nc.tensor.matmul(
                            ps[tt * 2 + fg2][:],
                            hT[:, dj, tt * 128:(tt + 1) * 128],
                            wt[:, fg2 * 512:(fg2 + 1) * 512],
                            start=(dj == 0), stop=(dj == 15))
            for tt in range(4):
                for fg2 in range(2):
                    nc.vector.tensor_copy(
                        vacc[tt][:, fgp * 1024 + fg2 * 512:
                                 fgp * 1024 + fg2 * 512 + 512],
                        ps[tt * 2 + fg2][:])

        def dump_k(half):
            nc.sync.dma_start(
                out=bass.AP(tensor=k_bounce[half].tensor,
                            offset=k_bounce[half].offset,
                            ap=[[512, 128], [128 * 512, 8], [1, 512]]),
                in_=kacc[:, half * 8:(half + 1) * 8, :])

        def dump_v(fgp):
            for tt in range(4):
                nc.sync.dma_start(
                    out=bass.AP(tensor=v_bounce[fgp].tensor,
                                offset=v_bounce[fgp].offset + tt * 128 * 1024,
                                ap=[[1024, 128], [1, 1024]]),
                    in_=vacc[tt][:, fgp * 1024:(fgp + 1) * 1024])

        import bass_rust as _br
        _cc_prev = [None]

        def ag(src, dst):
            cc = nc.gpsimd.collective_compute(
                "AllGather", OP.bypass, replica_groups=[list(range(NC))],
                ins=[src[:]], outs=[dst[:]])
            if _cc_prev[0] is not None:
                _br.add_dep_helper(cc.ins, _cc_prev[0].ins, sync=False,
                                   reason="cc issue order")
            _cc_prev[0] = cc

        proj_fmajor(wkT_d, 0, kacc)
        dump_k(0)
        ag(k_bounce[0], k_gath[0])
        proj_v(0)
        dump_v(0)
        ag(v_bounce[0], v_gath[0])
        proj_fmajor(wkT_d, 1, kacc)
        dump_k(1)
        ag(k_bounce[1], k_gath[1])
        proj_v(1)
        dump_v(1)
        ag(v_bounce[1], v_gath[1])
        wprep_emit()
        ag(wfc_bounce, wfc_gath)
        ag(wpj_bounce, wpj_gath)
        proj_fmajor(wqT_d, 0, QT)
        proj_fmajor(wqT_d, 1, QT)

        kacc_pool.__exit__(None, None, None)
        qkv_ps_pool.__exit__(None, None, None)
        qkv_pool.__exit__(None, None, None)

        # out-proj weight half dgp=0: prefetch + tanh during the
        # pre-attention K/V-gather window (fills the PE-idle gap and
        # keeps tanh out of the exp-bound attention span)
        KS = 2 if F8_OP else 1
        wo0_pool = tc.tile_pool(name="wo0", bufs=1)
        wo0p = wo0_pool.__enter__()
        wo0 = []
        with tc.tile_pool(name="wos", bufs=3) as wos:
            for og in range(16 // KS):
                raw = wos.tile([128, KS, 1024], dt.float32, tag="w0raw")
                nc.sync.dma_start(
                    out=raw[:],
                    in_=bass.AP(tensor=woT_d.tensor,
                                offset=woT_d.offset + og * KS * 128 * D,
                                ap=[[D, 128], [128 * D, KS], [1, 1024]]))
                wt0 = wo0p.tile([128, KS, 1024], op_dt, tag=f"wo0_{og}",
                                name=f"wo0_{og}")
                if F8_OP:
                    tnh = wos.tile([128, KS, 1024], dt.float32, tag="w0t32")
                    nc.scalar.activation(tnh[:], raw[:], AF.Tanh)
                    nc.vector.tensor_scalar(wt0[:], tnh[:], WS, None,
                                            op0=OP.mult)
                else:
                    nc.scalar.activation(wt0[:], raw[:], AF.Tanh)
                wo0.append(wt0)

        # ---------- Phase C: attention (causality via per-core masks) ----
        # rank r's gather block holds its 512 local tokens (256/batch,
        # rows {r, r+8, ...}); every core scans all 8 ranks per batch.
        OT = main.tile([128, 16, 512], op_dt, tag="bigA", bufs=2, name="OT")
        with tc.tile_pool(name="kvh", bufs=1) as kvh, \
             tc.tile_pool(name="att", bufs=8) as att, \
             tc.tile_pool(name="stps", bufs=4, space="PSUM") as stps, \
             tc.tile_pool(name="otps", bufs=2, space="PSUM") as otps, \
             tc.tile_pool(name="dnps", bufs=1, space="PSUM") as dnps, \
             tc.tile_pool(name="bcps", bufs=1, space="PSUM") as bcps:
            for hg in range(4):
                gh = hg // 2          # K/V gather half holding this hg
                ho = (hg % 2) * 4 * 128   # head offset inside the half
                kt_g, v_g = [], []
                for r in range(NC):
                    kt = kvh.tile([128, 4, 512], dt.bfloat16, tag="kth",
                                  bufs=11, name=f"kt_{hg}_{r}")
                    nc.sync.dma_start(
                        out=kt[:],
                        in_=bass.AP(
                            tensor=k_gath[gh].tensor,
                            offset=k_gath[gh].offset + r * KHALF
                            + ho * 512,
                            ap=[[512, 128], [128 * 512, 4], [1, 512]]))
                    kt_g.append(kt)
                    vt = kvh.tile([128, 4, 512], dt.bfloat16,
                                  tag="vth", bufs=11, name=f"vt_{hg}_{r}")
                    nc.sync.dma_start(
                        out=vt[:],
                        in_=bass.AP(
                            tensor=v_gath[gh].tensor,
                            offset=v_gath[gh].offset + r * VHALF + ho,
                            ap=[[1024, 128], [128 * 1024, 4], [1, 512]]))
                    v_g.append(vt)
                for b in range(2):
                    qoff = b * 256
                    for hh in range(4):
                        h = hg * 4 + hh
                        ot_ps = otps.tile([128, 256], dt.float32, tag="ot")
                        dn_ps = dnps.tile([1, 256], dt.float32, tag="dn")
                        for r in range(NC):
                            # st cols 0:256 = ksub0 x q[0:256],
                            #    cols 256:384 = ksub1 x q[128:256]
                            # (ksub1 is invisible to q[0:128] -> skipped)
                            st = stps.tile([128, 384], dt.float32, tag="st")
                            mm1 = nc.tensor.matmul(
                                st[:, 0:256],
                                kt_g[r][:, hh, qoff:qoff + 128],
                                QT[:, h, qoff:qoff + 256],
                                start=True, stop=False,
                                skip_group_check=True)
                            mm2 = nc.tensor.matmul(
                                st[:, 256:384],
                                kt_g[r][:, hh, qoff + 128:qoff + 256],
                                QT[:, h, qoff + 128:qoff + 256],
                                start=False, stop=False,
                                skip_group_check=True)
                            # mm1's bank-wide has_written clear must precede
                            # mm2 (regions are disjoint, Tile can't tell)
                            _br.add_dep_helper(mm2.ins, mm1.ins, sync=False,
                                               reason="st bank clear order")
                            nc.tensor.matmul(
                                st[:], ident_b[:], masks[:, r, :],
                                start=False, stop=True,
                                skip_group_check=True)
                            pt = att.tile([128, 384], dt.bfloat16, tag="pt")
                            nc.scalar.activation(pt[:], st[:], AF.Exp)
                            last = (r == NC - 1)
                            first = (r == 0)
                            nc.tensor.matmul(
                                ot_ps[:],
                                v_g[r][:, b * 2, hh * 128:(hh + 1) * 128],
                                pt[:, 0:256],
                                start=first, stop=False,
                                skip_group_check=True)
                            nc.tensor.matmul(
                                ot_ps[:, 128:256],
                                v_g[r][:, b * 2 + 1,
                                       hh * 128:(hh + 1) * 128],
                                pt[:, 256:384],
                                start=False, stop=last,
                                skip_group_check=True)
                            nc.tensor.matmul(
                                dn_ps[:], ones_col_b[:],
                                pt[:, 0:256],
                                start=first, stop=False,
                                skip_group_check=True)
                            nc.tensor.matmul(
                                dn_ps[:, 128:256], ones_col_b[:],
                                pt[:, 256:384],
                                start=False, stop=last,
                                skip_group_check=True)
                        dn_sb = att.tile([1, 256], dt.float32, tag="dns")
                        nc.vector.tensor_copy(dn_sb[:], dn_ps[:])
                        bc_ps = bcps.tile([128, 256], dt.float32, tag="bc")
                        nc.tensor.matmul(bc_ps[:], ones_row[:], dn_sb[:],
                                         start=True, stop=True)
                        rec_sb = att.tile([128, 256], dt.float32, tag="bcs")
                        nc.vector.reciprocal(rec_sb[:], bc_ps[:])
                        nc.vector.tensor_mul(OT[:, h, qoff:qoff + 256],
                                             ot_ps[:], rec_sb[:])

        # ---------- Phase D: out-proj + residual -> h2; LN2 -> m^T ----
        h2_pool = tc.tile_pool(name="h2a", bufs=1)
        h2a = h2_pool.__enter__()
        h2acc = [h2a.tile([128, D], dt.float32, name=f"h2_{t}")
                 for t in range(4)]
        with tc.tile_pool(name="wo", bufs=3) as wop, \
             tc.tile_pool(name="xd", bufs=3) as xd, \
             tc.tile_pool(name="dps", bufs=1, space="PSUM") as dps:
            for dgp in range(2):
                ps = [dps.tile([128, 512], dt.float32, tag=f"d{i}",
                               name=f"dp_{dgp}_{i}") for i in range(8)]
                for og in range(16 // KS):
                    if dgp == 0:
                        wt = wo0[og]
                    elif F8_OP:
                        raw = wop.tile([128, KS, 1024], dt.float32,
                                       tag="oraw")
                        nc.sync.dma_start(
                            out=raw[:],
                            in_=bass.AP(
                                tensor=woT_d.tensor,
                                offset=woT_d.offset
                                + og * KS * 128 * D + dgp * 1024,
                                ap=[[D, 128], [128 * D, KS], [1, 1024]]))
                        tnh = wop.tile([128, KS, 1024], dt.float32,
                                       tag="otnh32")
                        nc.scalar.activation(tnh[:], raw[:], AF.Tanh)
                        wt = wop.tile([128, KS, 1024], op_dt, tag="otnh")
                        nc.vector.tensor_scalar(wt[:], tnh[:], WS, None,
                                                op0=OP.mult)
                    else:
                        raw = wop.tile([128, KS, 1024], dt.float32,
                                       tag="oraw")
                        nc.sync.dma_start(
                            out=raw[:],
                            in_=bass.AP(
                                tensor=woT_d.tensor,
                                offset=woT_d.offset
                                + og * KS * 128 * D + dgp * 1024,
                                ap=[[D, 128], [128 * D, KS], [1, 1024]]))
                        wt = wop.tile([128, KS, 1024], dt.bfloat16,
                                      tag="otnh")
                        nc.scalar.activation(wt[:], raw[:], AF.Tanh)
                    for tt in range(4):
                        for dg2 in range(2):
                            if F8_OP:
                                nc.tensor.matmul(
                                    ps[tt * 2 + dg2][:],
                                    OT[:, og * 2:og * 2 + 2,
                                       tt * 128:(tt + 1) * 128],
                                    wt[:, :, dg2 * 512:(dg2 + 1) * 512],
                                    start=(og == 0), stop=(og == 7),
                                    perf_mode=DR)
                            else:
                                nc.tensor.matmul(
                                    ps[tt * 2 + dg2][:],
                                    OT[:, og, tt * 128:(tt + 1) * 128],
                                    wt[:, 0, dg2 * 512:(dg2 + 1) * 512],
                                    start=(og == 0), stop=(og == 15))
                for tt in range(4):
                    x_t = xd.tile([128, 1024], dt.float32, tag="x2")
                    nc.sync.dma_start(
                        out=x_t[:],
                        in_=xl_d[tt * 128:(tt + 1) * 128,
                                 dgp * 1024:(dgp + 1) * 1024])
                    for dg2 in range(2):
                        sl = slice(dgp * 1024 + dg2 * 512,
                                   dgp * 1024 + dg2 * 512 + 512)
                        if F8_OP:
                            nc.vector.tensor_scalar(
                                h2acc[tt][:, sl], ps[tt * 2 + dg2][:],
                                1.0 / WS, None, op0=OP.mult)
                            nc.vector.tensor_add(h2acc[tt][:, sl],
                                                 h2acc[tt][:, sl],
                                                 bo_bc[:, sl])
                        else:
                            nc.vector.tensor_add(h2acc[tt][:, sl],
                                                 ps[tt * 2 + dg2][:],
                                                 bo_bc[:, sl])
                        nc.vector.tensor_add(
                            h2acc[tt][:, sl], h2acc[tt][:, sl],
                            x_t[:, dg2 * 512:(dg2 + 1) * 512])

        mT = main.tile([128, 16, 512], mlp_dt, tag="bigA", bufs=2, name="mT")
        with tc.tile_pool(name="md", bufs=2) as md, \
             tc.tile_pool(name="trps2", bufs=4, space="PSUM") as trps2:
            for tb in range(4):
                m_t = md.tile([128, D], dt.float32, tag="m")
                layernorm(h2acc[tb], m_t, "g2", "b2")
                for dj in range(16):
                    ps = trps2.tile([128, 128], dt.float32, tag="tp2")
                    nc.tensor.transpose(ps[:], m_t[:, dj * 128:(dj + 1) * 128],
                                        ident[:])
                    nc.vector.tensor_copy(mT[:, dj, tb * 128:(tb + 1) * 128],
                                          ps[:])

        # ---------- Phase E: MLP (fp8 DoubleRow) ----------
        gt_pool = tc.tile_pool(name="gtpl", bufs=1)
        gtpl = gt_pool.__enter__()
        GT = gtpl.tile([128, 64, 512], mlp_dt, name="GT")
        wfcT_v = wfc_gath
        wpjT_v = wpj_gath
        MKS = 2 if F8_MLP else 1

        with tc.tile_pool(name="wfc", bufs=6) as wfcp, \
             tc.tile_pool(name="ups", bufs=2, space="PSUM") as ups:
            for grp in range(16):        # 4 f-tiles (512 features) per group
                ps = [ups.tile([128, 512], dt.float32, tag=f"u{i}",
                               name=f"u_{grp}_{i}") for i in range(4)]
                for djp in range(16 // MKS):
                    w2 = wfcp.tile([128, MKS, 512], mlp_dt, tag="wfct")
                    nc.sync.dma_start(
                        out=w2[:],
                        in_=bass.AP(tensor=wfcT_v.tensor,
                                    offset=wfcT_v.offset
                                    + djp * MKS * 128 * FF + grp * 512,
                                    ap=[[FF, 128], [128 * FF, MKS],
                                        [1, 512]]))
                    for f4 in range(4):
                        if F8_MLP:
                            nc.tensor.matmul(
                                ps[f4][:],
                                w2[:, :, f4 * 128:(f4 + 1) * 128],
                                mT[:, djp * 2:djp * 2 + 2, :],
                                start=(djp == 0), stop=(djp == 7),
                                perf_mode=DR)
                        else:
                            nc.tensor.matmul(
                                ps[f4][:],
                                w2[:, 0, f4 * 128:(f4 + 1) * 128],
                                mT[:, djp, :],
                                start=(djp == 0), stop=(djp == 15))
                for f4 in range(4):
                    fti = grp * 4 + f4
                    nc.scalar.activation(GT[:, fti, :], ps[f4][:],
                                         AF.Gelu_apprx_tanh,
                                         bias=bfc_pp[:, fti:fti + 1],
                                         scale=(1.0 / WS if F8_MLP else 1.0))

        with tc.tile_pool(name="wpj", bufs=4) as wpjp, \
             tc.tile_pool(name="yps", bufs=1, space="PSUM") as yps, \
             tc.tile_pool(name="outp", bufs=4) as outp:
            for ttp in range(2):
                ps = [yps.tile([128, 512], dt.float32, tag=f"y{i}",
                               name=f"y_{ttp}_{i}") for i in range(8)]
                for fp in range(64 // MKS):
                    wp2 = wpjp.tile([128, MKS, 2048], mlp_dt, tag="wpjt")
                    nc.sync.dma_start(
                        out=wp2[:],
                        in_=bass.AP(tensor=wpjT_v.tensor,
                                    offset=wpjT_v.offset
                                    + fp * MKS * 128 * D,
                                    ap=[[D, 128], [128 * D, MKS],
                                        [1, 2048]]))
                    for tt2 in range(2):
                        tt = ttp * 2 + tt2
                        for dg in range(4):
                            if F8_MLP:
                                nc.tensor.matmul(
                                    ps[tt2 * 4 + dg][:],
                                    GT[:, fp * 2:fp * 2 + 2,
                                       tt * 128:(tt + 1) * 128],
                                    wp2[:, :, dg * 512:(dg + 1) * 512],
                                    start=(fp == 0), stop=(fp == 31),
                                    perf_mode=DR)
                            else:
                                nc.tensor.matmul(
                                    ps[tt2 * 4 + dg][:],
                                    GT[:, fp, tt * 128:(tt + 1) * 128],
                                    wp2[:, 0, dg * 512:(dg + 1) * 512],
                                    start=(fp == 0), stop=(fp == 63))
                for tt2 in range(2):
                    tt = ttp * 2 + tt2
                    for dg in range(4):
                        sl = slice(dg * 512, dg * 512 + 512)
                        o_t = outp.tile([128, 512], dt.float32, tag="o")
                        if F8_MLP:
                            nc.vector.tensor_scalar(
                                o_t[:], ps[tt2 * 4 + dg][:], 1.0 / WS, None,
                                op0=OP.mult)
                            nc.vector.tensor_add(o_t[:], o_t[:],
                                                 bpj_bc[:, sl])
                        else:
                            nc.vector.tensor_add(o_t[:], ps[tt2 * 4 + dg][:],
                                                 bpj_bc[:, sl])
                        nc.vector.tensor_add(o_t[:], o_t[:],
                                             h2acc[tt][:, sl])
                        nc.sync.dma_start(
                            out=out_d[tt * 128:(tt + 1) * 128, sl],
                            in_=o_t[:])
        gt_pool.__exit__(None, None, None)
        h2_pool.__exit__(None, None, None)
        wo0_pool.__exit__(None, None, None)
        stack.close()

    nc.compile()
    return nc


def _host_prep(inputs):
    f32 = lambda k: np.ascontiguousarray(np.asarray(inputs[k], np.float32))
    x = f32("hidden_states")
    wqT = np.ascontiguousarray(f32("wq").T)
    wkT = np.ascontiguousarray(f32("wk").T)
    wvT = np.ascontiguousarray(f32("wv").T)
    woT = np.ascontiguousarray(f32("wo").T)
    wfcT = np.ascontiguousarray(f32("w_fc").T).ravel()
    wpjT = np.ascontiguousarray(f32("w_proj").T).ravel()
    # causal masks per core: q token = 8*qf + c, k token = 8*(ks*128+kp) + r
    # packed [128, 8, 384]: cols 0:256 = ksub0 x q[0:256],
    #                       cols 256:384 = ksub1 x q[128:256]
    kp = np.arange(128)[:, None, None]
    rr = np.arange(8)[None, :, None]
    ks = np.concatenate([np.zeros(256, np.int64),
                         np.ones(128, np.int64)])[None, None, :]
    qf = np.concatenate([np.arange(256),
                         np.arange(128, 256)])[None, None, :]
    in_maps = []
    for c in range(NC):
        mask = np.where(8 * (ks * 128 + kp) + rr <= 8 * qf + c,
                        0.0, -1e9).astype(np.float32)
        mask = mask.astype(ml_dtypes.bfloat16)
        in_maps.append({
            "xl": np.concatenate([x[0, c::NC, :], x[1, c::NC, :]], 0),
            "wqT": wqT, "wkT": wkT, "wvT": wvT, "woT": woT,
            "wfc_ch": wfcT[c * WFC_CH:(c + 1) * WFC_CH],
            "wpj_ch": wpjT[c * WPJ_CH:(c + 1) * WPJ_CH],
            "mask": mask,
            "ln1g": f32("ln1_g"), "ln1b": f32("ln1_b"),
            "ln2g": f32("ln2_g"), "ln2b": f32("ln2_b"),
            "bo": f32("bo"), "bfc": f32("b_fc"), "bpj": f32("b_proj"),
        })
    return in_maps


def kernel(**inputs) -> np.ndarray:
    in_maps = _host_prep(inputs)
    key = (not bool(np.all(np.asarray(inputs["ln1_g"]) == 1.0)),
           not bool(np.all(np.asarray(inputs["ln1_b"]) == 0.0)),
           not bool(np.all(np.asarray(inputs["ln2_g"]) == 1.0)),
           not bool(np.all(np.asarray(inputs["ln2_b"]) == 0.0)))
    if key not in _CACHE:
        _CACHE[key] = _build(*key)
    nc = _CACHE[key]
    res = run_bass_kernel_spmd(nc, in_maps, core_ids=list(range(NC)))
    if res.exec_time_ns is not None:
        print(f"HW exec time: {res.exec_time_ns} ns")
    out = np.zeros((B, S, D), np.float32)
    for c in range(NC):
        o = res.results[c]["out"]
        out[0, c::NC] = o[:CH]
        out[1, c::NC] = o[CH:]
    return out


# revision 47
# speedup vs baseline: 1.0548x; 1.0548x over previous
"""BinaryGPTNeoBlock on 8 trn2 NeuronCores.

Sequence-parallel over 8 cores: core c owns rows {c, c+8, ...} of both
batches (256 per batch, 512 total); causality is per-core mask data so
the program stays SPMD-uniform. K/V are projected feature-/token-major
directly (no PE transposes), cast bf16, and AllGathered in two halves
each (interleaved with the projection passes) so attention starts with
no stall. MLP weights: each core tanh's + scales (x64) its 1/8th into
fp8, two AllGathers share them, and fc/proj run fp8 DoubleRow matmuls
(2x PE rate); the 1/64 descale folds into PSUM evacuation.

Self-contained: hardcodes shapes; host only shards/transposes/builds masks.
"""

import numpy as np
import ml_dtypes

import concourse.bass as bass
import concourse.tile as tile
from concourse import bacc, mybir
from concourse.bass_utils import run_bass_kernel_spmd
from concourse.masks import make_identity

B, S, D = 2, 2048, 2048
H = 16
HD = 128
FF = 4 * D
EPS = 1e-5
NC = 8
CH = 256               # q-chunk length (S // NC)
TL = 2 * CH            # 512 local rows (one chunk per batch)
WFC_CH = 256 * FF      # own d-rows of wfcT
WPJ_CH = 1024 * D      # own f-rows of wpjT
WS = 64.0              # fp8 weight pre-scale (undone at PSUM evacuation)

F8_MLP = True          # fc/proj in fp8 DoubleRow
F8_OP = False          # out-proj in fp8 DoubleRow

dt = mybir.dt
AF = mybir.ActivationFunctionType
OP = mybir.AluOpType
DR = mybir.MatmulPerfMode.DoubleRow

_CACHE = {}


def _build(apply_g1, apply_b1, apply_g2, apply_b2):
    nc = bacc.Bacc("TRN2", target_bir_lowering=False, debug=False,
                   num_devices=NC)

    xl_d = nc.dram_tensor("xl", [TL, D], dt.float32, kind="ExternalInput").ap()
    wqT_d = nc.dram_tensor("wqT", [D, D], dt.float32, kind="ExternalInput").ap()
    wkT_d = nc.dram_tensor("wkT", [D, D], dt.float32, kind="ExternalInput").ap()
    wvT_d = nc.dram_tensor("wvT", [D, D], dt.float32, kind="ExternalInput").ap()
    woT_d = nc.dram_tensor("woT", [D, D], dt.float32, kind="ExternalInput").ap()
    wfc_ch_d = nc.dram_tensor("wfc_ch", [WFC_CH], dt.float32,
                              kind="ExternalInput").ap()
    wpj_ch_d = nc.dram_tensor("wpj_ch", [WPJ_CH], dt.float32,
                              kind="ExternalInput").ap()
    mask_d = nc.dram_tensor("mask", [128, 8, 384], dt.bfloat16,
                            kind="ExternalInput").ap()
    ln1g_d = nc.dram_tensor("ln1g", [D], dt.float32, kind="ExternalInput").ap()
    ln1b_d = nc.dram_tensor("ln1b", [D], dt.float32, kind="ExternalInput").ap()
    ln2g_d = nc.dram_tensor("ln2g", [D], dt.float32, kind="ExternalInput").ap()
    ln2b_d = nc.dram_tensor("ln2b", [D], dt.float32, kind="ExternalInput").ap()
    bo_d = nc.dram_tensor("bo", [D], dt.float32, kind="ExternalInput").ap()
    bfc_d = nc.dram_tensor("bfc", [FF], dt.float32, kind="ExternalInput").ap()
    bpj_d = nc.dram_tensor("bpj", [D], dt.float32, kind="ExternalInput").ap()
    out_d = nc.dram_tensor("out", [TL, D], dt.float32,
                           kind="ExternalOutput").ap()

    mlp_dt = dt.float8e4 if F8_MLP else dt.bfloat16
    op_dt = dt.float8e4 if F8_OP else dt.bfloat16
    KHALF = 1024 * TL        # elems of one kT half per core
    VHALF = TL * 1024

    def bcast_row(src_ap, n):
        return bass.AP(tensor=src_ap.tensor, offset=src_ap.offset,
                       ap=[[0, 128], [1, n]])

    with tile.TileContext(nc) as tc:
        import contextlib
        stack = contextlib.ExitStack()
        main = stack.enter_context(tc.tile_pool(name="main", bufs=1))
        dram = stack.enter_context(
            tc.tile_pool(name="dram", bufs=1, space="DRAM"))

        ident = main.tile([128, 128], dt.float32)
        make_identity(nc, ident[:])
        ident_b = main.tile([128, 128], dt.bfloat16)
        nc.vector.tensor_copy(ident_b[:], ident[:])
        ones_col_b = main.tile([128, 1], dt.bfloat16)
        nc.vector.memset(ones_col_b[:], 1.0)
        ones_row = main.tile([1, 128], dt.float32)
        nc.vector.memset(ones_row[:], 1.0)
        eps_t = main.tile([128, 1], dt.float32)
        nc.vector.memset(eps_t[:], EPS)
        bo_bc = main.tile([128, D], dt.float32)
        nc.sync.dma_start(out=bo_bc[:], in_=bcast_row(bo_d, D))
        bpj_bc = main.tile([128, D], dt.float32)
        nc.sync.dma_start(out=bpj_bc[:], in_=bcast_row(bpj_d, D))
        masks = main.tile([128, 8, 384], dt.bfloat16)
        nc.sync.dma_start(out=masks[:], in_=mask_d[:])
        bfc_pp = main.tile([128, FF // 128], dt.float32)
        nc.sync.dma_start(
            out=bfc_pp[:],
            in_=bass.AP(tensor=bfc_d.tensor, offset=bfc_d.offset,
                        ap=[[1, 128], [128, FF // 128]]))
        ln_bc = {}
        for nm, flag, src in (("g1", apply_g1, ln1g_d),
                              ("b1", apply_b1, ln1b_d),
                              ("g2", apply_g2, ln2g_d),
                              ("b2", apply_b2, ln2b_d)):
            if flag:
                t = main.tile([128, D], dt.float32, name=f"ln_{nm}")
                nc.sync.dma_start(out=t[:], in_=bcast_row(src, D))
                ln_bc[nm] = t

        # rotating big activation slots (16KB/part each, 2 slots)
        hT = main.tile([128, 16, 512], dt.bfloat16, tag="bigA", bufs=2,
                       name="hT")
        QT = main.tile([128, 16, 512], dt.bfloat16, tag="bigA", bufs=2,
                       name="QT")

        wfc_bounce = dram.tile([WFC_CH], mlp_dt)
        wpj_bounce = dram.tile([WPJ_CH], mlp_dt)
        wfc_gath = dram.tile([NC * WFC_CH], mlp_dt, addr_space="Shared")
        wpj_gath = dram.tile([NC * WPJ_CH], mlp_dt, addr_space="Shared")

        def wprep_emit():
            # tanh + x64 + fp8-cast of own 1/8th of the MLP weights
            with tc.tile_pool(name="wprep", bufs=3) as wprep:
                for src, dst, nrb, ncols in ((wfc_ch_d, wfc_bounce, 2, FF),
                                             (wpj_ch_d, wpj_bounce, 8, D)):
                    for rb in range(nrb):
                        for ci in range(ncols // 2048):
                            off = rb * 128 * ncols + ci * 2048
                            raw = wprep.tile([128, 2048], dt.float32,
                                             tag="wraw")
                            nc.sync.dma_start(
                                out=raw[:],
                                in_=bass.AP(tensor=src.tensor,
                                            offset=src.offset + off,
                                            ap=[[ncols, 128], [1, 2048]]))
                            tnh = wprep.tile([128, 2048], dt.float32,
                                             tag="wtnh")
                            nc.scalar.activation(tnh[:], raw[:], AF.Tanh)
                            w8 = wprep.tile([128, 2048], mlp_dt, tag="w8")
                            if F8_MLP:
                                nc.vector.tensor_scalar(w8[:], tnh[:], WS,
                                                        None, op0=OP.mult)
                            else:
                                nc.vector.tensor_copy(w8[:], tnh[:])
                            nc.sync.dma_start(
                                out=bass.AP(tensor=dst.tensor,
                                            offset=dst.offset + off,
                                            ap=[[ncols, 128], [1, 2048]]),
                                in_=w8[:])

        # ---------- Phase A: x -> LN1 -> h^T ----------
        def layernorm(x_t, h_t, gk, bk):
            with tc.tile_pool(name="lnp", bufs=2) as lp:
                st = lp.tile([128, 4, 6], dt.float32, tag="st")
                xr = x_t[:].rearrange("p (n f) -> p n f", n=4)
                for sg in range(4):
                    nc.vector.bn_stats(out=st[:, sg, :], in_=xr[:, sg, :])
                mv = lp.tile([128, 2], dt.float32, tag="mv")
                nc.vector.bn_aggr(out=mv[:], in_=st[:])
                std = lp.tile([128, 1], dt.float32, tag="sd")
                nc.scalar.activation(std[:], mv[:, 1:2], AF.Sqrt,
                                     bias=eps_t[:])
                rstd = lp.tile([128, 1], dt.float32, tag="rs")
                nc.vector.reciprocal(rstd[:], std[:])
                nc.vector.tensor_scalar(h_t[:], x_t[:], mv[:, 0:1], rstd[:],
                                        op0=OP.subtract, op1=OP.mult)
                if gk in ln_bc:
                    nc.vector.tensor_mul(h_t[:], h_t[:], ln_bc[gk][:])
                if bk in ln_bc:
                    nc.vector.tensor_add(h_t[:], h_t[:], ln_bc[bk][:])

        with tc.tile_pool(name="xa", bufs=2) as xa, \
             tc.tile_pool(name="ha", bufs=1) as ha, \
             tc.tile_pool(name="trps", bufs=4, space="PSUM") as trps:
            h_ts = []
            for tb in range(4):
                x_t = xa.tile([128, D], dt.float32, tag="x")
                nc.sync.dma_start(out=x_t[:],
                                  in_=xl_d[tb * 128:(tb + 1) * 128, :])
                h_t = ha.tile([128, D], dt.float32, tag=f"h{tb}",
                              name=f"h_{tb}")
                layernorm(x_t, h_t, "g1", "b1")
                h_ts.append(h_t)
            # dj-major so hT[:, dj, :] completes early -> K matmuls overlap
            for dj in range(16):
                for tb in range(4):
                    ps = trps.tile([128, 128], dt.float32, tag="tp")
                    nc.tensor.transpose(
                        ps[:], h_ts[tb][:, dj * 128:(dj + 1) * 128],
                        ident[:])
                    nc.vector.tensor_copy(hT[:, dj, tb * 128:(tb + 1) * 128],
                                          ps[:])

        # ---------- Phase B: QKV (feature-major K/Q, token-major V) ----
        k_bounce = [dram.tile([KHALF], dt.bfloat16, name=f"kb{i}")
                    for i in range(2)]
        v_bounce = [dram.tile([VHALF], dt.bfloat16, name=f"vb{i}")
                    for i in range(2)]
        k_gath = [dram.tile([NC * KHALF], dt.bfloat16, addr_space="Shared",
                            name=f"kg{i}") for i in range(2)]
        v_gath = [dram.tile([NC * VHALF], dt.bfloat16, addr_space="Shared",
                            name=f"vg{i}") for i in range(2)]

        qkv_pool = tc.tile_pool(name="qkv", bufs=3)
        qkvp = qkv_pool.__enter__()
        qkv_ps_pool = tc.tile_pool(name="qkvps", bufs=1, space="PSUM")
        qkvps = qkv_ps_pool.__enter__()
        kacc_pool = tc.tile_pool(name="kacc", bufs=1)
        kaccp = kacc_pool.__enter__()
        kacc = kaccp.tile([128, 16, 512], dt.bfloat16, name="kacc")
        vacc = [kaccp.tile([128, D], dt.bfloat16, name=f"vacc{t}")
                for t in range(4)]

        def proj_fmajor(wT_dram, pss, dest):
            # features [pss*1024, pss*1024+1024) of w^T h^T -> dest[:, 8pss..]
            ps = [qkvps.tile([128, 512], dt.float32, tag=f"q{i}",
                             name=f"ps_{wT_dram.tensor.name}_{pss}_{i}")
                  for i in range(8)]
            for dj in range(16):
                raw = qkvp.tile([128, 1024], dt.float32, tag="qkraw", bufs=6)
                nc.sync.dma_start(
                    out=raw[:],
                    in_=wT_dram[dj * 128:(dj + 1) * 128,
                                pss * 1024:(pss + 1) * 1024])
                wt = qkvp.tile([128, 1024], dt.bfloat16, tag="qktnh", bufs=4)
                nc.scalar.activation(wt[:], raw[:], AF.Tanh)
                for ft in range(8):
                    nc.tensor.matmul(ps[ft][:],
                                     wt[:, ft * 128:(ft + 1) * 128],
                                     hT[:, dj, :],
                                     start=(dj == 0), stop=(dj == 15))
            for ft in range(8):
                nc.vector.tensor_copy(dest[:, pss * 8 + ft, :], ps[ft][:])

        def proj_v(fgp):
            # token-major v for features [fgp*1024, fgp*1024+1024)
            ps = [qkvps.tile([128, 512], dt.float32, tag=f"q{i}",
                             name=f"ps_v_{fgp}_{i}")
                  for i in range(8)]
            for dj in range(16):
                raw = qkvp.tile([128, 1024], dt.float32, tag="qkraw", bufs=6)
                nc.sync.dma_start(
                    out=raw[:],
                    in_=wvT_d[dj * 128:(dj + 1) * 128,
                              fgp * 1024:(fgp + 1) * 1024])
                wt = qkvp.tile([128, 1024], dt.bfloat16, tag="qktnh", bufs=4)
                nc.scalar.activation(wt[:], raw[:], AF.Tanh)
                for tt in range(4):
                    for fg2 in range(2):
                        nc.tensor.matmul(
                            ps[tt * 2 + fg2][:],
                            hT[:, dj, tt * 128:(tt + 1) * 128],
                            wt[:, fg2 * 512:(fg2 + 1) * 512],
                            start=(dj == 0), stop=(dj == 15))
            for tt in range(4):
                for fg2 in range(2):
                    nc.vector.tensor_copy(
                        vacc[tt][:, fgp * 1024 + fg2 * 512:
                                 fgp * 1024 + fg2 * 512 + 512],
                        ps[tt * 2 + fg2][:])

        def dump_k(half):
            nc.sync.dma_start(
                out=bass.AP(tensor=k_bounce[half].tensor,
                            offset=k_bounce[half].offset,
                            ap=[[512, 128], [128 * 512, 8], [1, 512]]),
                in_=kacc[:, half * 8:(half + 1) * 8, :])

        def dump_v(fgp):
            for tt in range(4):
                nc.sync.dma_start(
                    out=bass.AP(tensor=v_bounce[fgp].tensor,
                                offset=v_bounce[fgp].offset + tt * 128 * 1024,
                                ap=[[1024, 128], [1, 1024]]),
                    in_=vacc[tt][:, fgp * 1024:(fgp + 1) * 1024])

        import bass_rust as _br
        _cc_prev = [None]

        def ag(src, dst):
            cc = nc.gpsimd.collective_compute(
                "AllGather", OP.bypass, replica_groups=[list(range(NC))],
                ins=[src[:]], outs=[dst[:]])
            if _cc_prev[0] is not None:
                _br.add_dep_helper(cc.ins, _cc_prev[0].ins, sync=False,
                                   reason="cc issue order")
            _cc_prev[0] = cc

        proj_fmajor(wkT_d, 0, kacc)
        dump_k(0)
        ag(k_bounce[0], k_gath[0])
        proj_v(0)
        dump_v(0)
        ag(v_bounce[0], v_gath[0])
        proj_fmajor(wkT_d, 1, kacc)
        dump_k(1)
        ag(k_bounce[1], k_gath[1])
        proj_v(1)
        dump_v(1)
        ag(v_bounce[1], v_gath[1])
        wprep_emit()
        ag(wfc_bounce, wfc_gath)
        ag(wpj_bounce, wpj_gath)
        proj_fmajor(wqT_d, 0, QT)
        proj_fmajor(wqT_d, 1, QT)

        kacc_pool.__exit__(None, None, None)
        qkv_ps_pool.__exit__(None, None, None)
        qkv_pool.__exit__(None, None, None)

        # ---------- Phase C: attention (causality via per-core masks) ----
        # rank r's gather block holds its 512 local tokens (256/batch,
        # rows {r, r+8, ...}); every core scans all 8 ranks per batch.
        OT = main.tile([128, 16, 512], op_dt, tag="bigA", bufs=2, name="OT")
        with tc.tile_pool(name="kvh", bufs=1) as kvh, \
             tc.tile_pool(name="att", bufs=8) as att, \
             tc.tile_pool(name="stps", bufs=4, space="PSUM") as stps, \
             tc.tile_pool(name="otps", bufs=2, space="PSUM") as otps, \
             tc.tile_pool(name="dnps", bufs=1, space="PSUM") as dnps, \
             tc.tile_pool(name="bcps", bufs=1, space="PSUM") as bcps:
            for hg in range(4):
                gh = hg // 2          # K/V gather half holding this hg
                ho = (hg % 2) * 4 * 128   # head offset inside the half
                kt_g, v_g = [], []
                for r in range(NC):
                    kt = kvh.tile([128, 4, 512], dt.bfloat16, tag="kth",
                                  bufs=11, name=f"kt_{hg}_{r}")
                    nc.sync.dma_start(
                        out=kt[:],
                        in_=bass.AP(
                            tensor=k_gath[gh].tensor,
                            offset=k_gath[gh].offset + r * KHALF
                            + ho * 512,
                            ap=[[512, 128], [128 * 512, 4], [1, 512]]))
                    kt_g.append(kt)
                    vt = kvh.tile([128, 4, 512], dt.bfloat16,
                                  tag="vth", bufs=11, name=f"vt_{hg}_{r}")
                    nc.sync.dma_start(
                        out=vt[:],
                        in_=bass.AP(
                            tensor=v_gath[gh].tensor,
                            offset=v_gath[gh].offset + r * VHALF + ho,
                            ap=[[1024, 128], [128 * 1024, 4], [1, 512]]))
                    v_g.append(vt)
                for b in range(2):
                    qoff = b * 256
                    for hh in range(4):
                        h = hg * 4 + hh
                        ot_ps = otps.tile([128, 256], dt.float32, tag="ot")
                        dn_ps = dnps.tile([1, 256], dt.float32, tag="dn")
                        for r in range(NC):
                            # st cols 0:256 = ksub0 x q[0:256],
                            #    cols 256:384 = ksub1 x q[128:256]
                            # (ksub1 is invisible to q[0:128] -> skipped)
                            st = stps.tile([128, 384], dt.float32, tag="st")
                            mm1 = nc.tensor.matmul(
                                st[:, 0:256],
                                kt_g[r][:, hh, qoff:qoff + 128],
                                QT[:, h, qoff:qoff + 256],
                                start=True, stop=False,
                                skip_group_check=True)
                            mm2 = nc.tensor.matmul(
                                st[:, 256:384],
                                kt_g[r][:, hh, qoff + 128:qoff + 256],
                                QT[:, h, qoff + 128:qoff + 256],
                                start=False, stop=False,
                                skip_group_check=True)
                            # mm1's bank-wide has_written clear must precede
                            # mm2 (regions are disjoint, Tile can't tell)
                            _br.add_dep_helper(mm2.ins, mm1.ins, sync=False,
                                               reason="st bank clear order")
                            nc.tensor.matmul(
                                st[:], ident_b[:], masks[:, r, :],
                                start=False, stop=True,
                                skip_group_check=True)
                            pt = att.tile([128, 384], dt.bfloat16, tag="pt")
                            nc.scalar.activation(pt[:], st[:], AF.Exp)
                            last = (r == NC - 1)
                            first = (r == 0)
                            nc.tensor.matmul(
                                ot_ps[:],
                                v_g[r][:, b * 2, hh * 128:(hh + 1) * 128],
                                pt[:, 0:256],
                                start=first, stop=False,
                                skip_group_check=True)
                            nc.tensor.matmul(
                                ot_ps[:, 128:256],
                                v_g[r][:, b * 2 + 1,
                                       hh * 128:(hh + 1) * 128],
                                pt[:, 256:384],
                                start=False, stop=last,
                                skip_group_check=True)
                            nc.tensor.matmul(
                                dn_ps[:], ones_col_b[:],
                                pt[:, 0:256],
                                start=first, stop=False,
                                skip_group_check=True)
                            nc.tensor.matmul(
                                dn_ps[:, 128:256], ones_col_b[:],
                                pt[:, 256:384],
                                start=False, stop=last,
                                skip_group_check=True)
                        dn_sb = att.tile([1, 256], dt.float32, tag="dns")
                        nc.vector.tensor_copy(dn_sb[:], dn_ps[:])
                        bc_ps = bcps.tile([128, 256], dt.float32, tag="bc")
                        nc.tensor.matmul(bc_ps[:], ones_row[:], dn_sb[:],
                                         start=True, stop=True)
                        rec_sb = att.tile([128, 256], dt.float32, tag="bcs")
                        nc.vector.reciprocal(rec_sb[:], bc_ps[:])
                        nc.vector.tensor_mul(OT[:, h, qoff:qoff + 256],
                                             ot_ps[:], rec_sb[:])

        # ---------- Phase D: out-proj + residual -> h2; LN2 -> m^T ----
        h2_pool = tc.tile_pool(name="h2a", bufs=1)
        h2a = h2_pool.__enter__()
        h2acc = [h2a.tile([128, D], dt.float32, name=f"h2_{t}")
                 for t in range(4)]
        KS = 2 if F8_OP else 1
        with tc.tile_pool(name="wo", bufs=3) as wop, \
             tc.tile_pool(name="xd", bufs=3) as xd, \
             tc.tile_pool(name="dps", bufs=1, space="PSUM") as dps:
            for dgp in range(2):
                ps = [dps.tile([128, 512], dt.float32, tag=f"d{i}",
                               name=f"dp_{dgp}_{i}") for i in range(8)]
                for og in range(16 // KS):
                    raw = wop.tile([128, KS, 1024], dt.float32, tag="oraw")
                    nc.sync.dma_start(
                        out=raw[:],
                        in_=bass.AP(tensor=woT_d.tensor,
                                    offset=woT_d.offset
                                    + og * KS * 128 * D + dgp * 1024,
                                    ap=[[D, 128], [128 * D, KS], [1, 1024]]))
                    if F8_OP:
                        tnh = wop.tile([128, KS, 1024], dt.float32,
                                       tag="otnh32")
                        nc.scalar.activation(tnh[:], raw[:], AF.Tanh)
                        wt = wop.tile([128, KS, 1024], op_dt, tag="otnh")
                        nc.vector.tensor_scalar(wt[:], tnh[:], WS, None,
                                                op0=OP.mult)
                    else:
                        wt = wop.tile([128, KS, 1024], dt.bfloat16,
                                      tag="otnh")
                        nc.scalar.activation(wt[:], raw[:], AF.Tanh)
                    for tt in range(4):
                        for dg2 in range(2):
                            if F8_OP:
                                nc.tensor.matmul(
                                    ps[tt * 2 + dg2][:],
                                    OT[:, og * 2:og * 2 + 2,
                                       tt * 128:(tt + 1) * 128],
                                    wt[:, :, dg2 * 512:(dg2 + 1) * 512],
                                    start=(og == 0), stop=(og == 7),
                                    perf_mode=DR)
                            else:
                                nc.tensor.matmul(
                                    ps[tt * 2 + dg2][:],
                                    OT[:, og, tt * 128:(tt + 1) * 128],
                                    wt[:, 0, dg2 * 512:(dg2 + 1) * 512],
                                    start=(og == 0), stop=(og == 15))
                for tt in range(4):
                    x_t = xd.tile([128, 1024], dt.float32, tag="x2")
                    nc.sync.dma_start(
                        out=x_t[:],
                        in_=xl_d[tt * 128:(tt + 1) * 128,
                                 dgp * 1024:(dgp + 1) * 1024])
                    for dg2 in range(2):
                        sl = slice(dgp * 1024 + dg2 * 512,
                                   dgp * 1024 + dg2 * 512 + 512)
                        if F8_OP:
                            nc.vector.tensor_scalar(
                                h2acc[tt][:, sl], ps[tt * 2 + dg2][:],
                                1.0 / WS, None, op0=OP.mult)
                            nc.vector.tensor_add(h2acc[tt][:, sl],
                                                 h2acc[tt][:, sl],
                                                 bo_bc[:, sl])
                        else:
                            nc.vector.tensor_add(h2acc[tt][:, sl],
                                                 ps[tt * 2 + dg2][:],
                                                 bo_bc[:, sl])
                        nc.vector.tensor_add(
                            h2acc[tt][:, sl], h2acc[tt][:, sl],
                            x_t[:, dg2 * 512:(dg2 + 1) * 512])

        mT = main.tile([128, 16, 512], mlp_dt, tag="bigA", bufs=2, name="mT")
        with tc.tile_pool(name="md", bufs=2) as md, \
             tc.tile_pool(name="trps2", bufs=4, space="PSUM") as trps2:
            for tb in range(4):
                m_t = md.tile([128, D], dt.float32, tag="m")
                layernorm(h2acc[tb], m_t, "g2", "b2")
                for dj in range(16):
                    ps = trps2.tile([128, 128], dt.float32, tag="tp2")
                    nc.tensor.transpose(ps[:], m_t[:, dj * 128:(dj + 1) * 128],
                                        ident[:])
                    nc.vector.tensor_copy(mT[:, dj, tb * 128:(tb + 1) * 128],
                                          ps[:])

        # ---------- Phase E: MLP (fp8 DoubleRow) ----------
        gt_pool = tc.tile_pool(name="gtpl", bufs=1)
        gtpl = gt_pool.__enter__()
        GT = gtpl.tile([128, 64, 512], mlp_dt, name="GT")
        wfcT_v = wfc_gath
        wpjT_v = wpj_gath
        MKS = 2 if F8_MLP else 1

        with tc.tile_pool(name="wfc", bufs=6) as wfcp, \
             tc.tile_pool(name="ups", bufs=2, space="PSUM") as ups:
            for grp in range(16):        # 4 f-tiles (512 features) per group
                ps = [ups.tile([128, 512], dt.float32, tag=f"u{i}",
                               name=f"u_{grp}_{i}") for i in range(4)]
                for djp in range(16 // MKS):
                    w2 = wfcp.tile([128, MKS, 512], mlp_dt, tag="wfct")
                    nc.sync.dma_start(
                        out=w2[:],
                        in_=bass.AP(tensor=wfcT_v.tensor,
                                    offset=wfcT_v.offset
                                    + djp * MKS * 128 * FF + grp * 512,
                                    ap=[[FF, 128], [128 * FF, MKS],
                                        [1, 512]]))
                    for f4 in range(4):
                        if F8_MLP:
                            nc.tensor.matmul(
                                ps[f4][:],
                                w2[:, :, f4 * 128:(f4 + 1) * 128],
                                mT[:, djp * 2:djp * 2 + 2, :],
                                start=(djp == 0), stop=(djp == 7),
                                perf_mode=DR)
                        else:
                            nc.tensor.matmul(
                                ps[f4][:],
                                w2[:, 0, f4 * 128:(f4 + 1) * 128],
                                mT[:, djp, :],
                                start=(djp == 0), stop=(djp == 15))
                for f4 in range(4):
                    fti = grp * 4 + f4
                    nc.scalar.activation(GT[:, fti, :], ps[f4][:],
                                         AF.Gelu_apprx_tanh,
                                         bias=bfc_pp[:, fti:fti + 1],
                                         scale=(1.0 / WS if F8_MLP else 1.0))

        with tc.tile_pool(name="wpj", bufs=4) as wpjp, \
             tc.tile_pool(name="yps", bufs=1, space="PSUM") as yps, \
             tc.tile_pool(name="outp", bufs=4) as outp:
            for ttp in range(2):
                ps = [yps.tile([128, 512], dt.float32, tag=f"y{i}",
                               name=f"y_{ttp}_{i}") for i in range(8)]
                for fp in range(64 // MKS):
                    wp2 = wpjp.tile([128, MKS, 2048], mlp_dt, tag="wpjt")
                    nc.sync.dma_start(
                        out=wp2[:],
                        in_=bass.AP(tensor=wpjT_v.tensor,
                                    offset=wpjT_v.offset
                                    + fp * MKS * 128 * D,
                                    ap=[[D, 128], [128 * D, MKS],
                                        [1, 2048]]))
                    for tt2 in range(2):
                        tt = ttp * 2 + tt2
                        for dg in range(4):
                            if F8_MLP:
                                nc.tensor.matmul(
                                    ps[tt2 * 4 + dg][:],
                                    GT[:, fp * 2:fp * 2 + 2,
                                       tt * 128:(tt + 1) * 128],
                                    wp2[:, :, dg * 512:(dg + 1) * 512],
                                    start=(fp == 0), stop=(fp == 31),
                                    perf_mode=DR)
                            else:
                                nc.tensor.matmul(
                                    ps[tt2 * 4 + dg][:],
                                    GT[:, fp, tt * 128:(tt + 1) * 128],
                                    wp2[:, 0, dg * 512:(dg + 1) * 512],
                                    start=(fp == 0), stop=(fp == 63))
                for tt2 in range(2):
                    tt = ttp * 2 + tt2
                    for dg in range(4):
                        sl = slice(dg * 512, dg * 512 + 512)
                        o_t = outp.tile([128, 512], dt.float32, tag="o")
                        if F8_MLP:
                            nc.vector.tensor_scalar(
                                o_t[:], ps[tt2 * 4 + dg][:], 1.0 / WS, None,
                                op0=OP.mult)
                            nc.vector.tensor_add(o_t[:], o_t[:],
                                                 bpj_bc[:, sl])
                        else:
                            nc.vector.tensor_add(o_t[:], ps[tt2 * 4 + dg][:],
                                                 bpj_bc[:, sl])
                        nc.vector.tensor_add(o_t[:], o_t[:],
                                             h2acc[tt][:, sl])
                        nc.sync.dma_start(
                            out=out_d[tt * 128:(tt + 1) * 128, sl],
                            in_=o_t[:])
        gt_pool.__exit__(None, None, None)
        h2_pool.__exit__(None, None, None)
        stack.close()

    nc.compile()
    return nc


def _host_prep(inputs):
    f32 = lambda k: np.ascontiguousarray(np.asarray(inputs[k], np.float32))
    x = f32("hidden_states")
    wqT = np.ascontiguousarray(f32("wq").T)
    wkT = np.ascontiguousarray(f32("wk").T)
    wvT = np.ascontiguousarray(f32("wv").T)
    woT = np.ascontiguousarray(f32("wo").T)
    wfcT = np.ascontiguousarray(f32("w_fc").T).ravel()
    wpjT = np.ascontiguousarray(f32("w_proj").T).ravel()
    # causal masks per core: q token = 8*qf + c, k token = 8*(ks*128+kp) + r
    # packed [128, 8, 384]: cols 0:256 = ksub0 x q[0:256],
    #                       cols 256:384 = ksub1 x q[128:256]
    kp = np.arange(128)[:, None, None]
    rr = np.arange(8)[None, :, None]
    ks = np.concatenate([np.zeros(256, np.int64),
                         np.ones(128, np.int64)])[None, None, :]
    qf = np.concatenate([np.arange(256),
                         np.arange(128, 256)])[None, None, :]
    in_maps = []
    for c in range(NC):
        mask = np.where(8 * (ks * 128 + kp) + rr <= 8 * qf + c,
                        0.0, -1e9).astype(np.float32)
        mask = mask.astype(ml_dtypes.bfloat16)
        in_maps.append({
            "xl": np.concatenate([x[0, c::NC, :], x[1, c::NC, :]], 0),
            "wqT": wqT, "wkT": wkT, "wvT": wvT, "woT": woT,
            "wfc_ch": wfcT[c * WFC_CH:(c + 1) * WFC_CH],
            "wpj_ch": wpjT[c * WPJ_CH:(c + 1) * WPJ_CH],
            "mask": mask,
            "ln1g": f32("ln1_g"), "ln1b": f32("ln1_b"),
            "ln2g": f32("ln2_g"), "ln2b": f32("ln2_b"),
            "bo": f32("bo"), "bfc": f32("b_fc"), "bpj": f32("b_proj"),
        })
    return in_maps


def kernel(**inputs) -> np.ndarray:
    in_maps = _host_prep(inputs)
    key = (not bool(np.all(np.asarray(inputs["ln1_g"]) == 1.0)),
           not bool(np.all(np.asarray(inputs["ln1_b"]) == 0.0)),
           not bool(np.all(np.asarray(inputs["ln2_g"]) == 1.0)),
           not bool(np.all(np.asarray(inputs["ln2_b"]) == 0.0)))
    if key not in _CACHE:
        _CACHE[key] = _build(*key)
    nc = _CACHE[key]
    res = run_bass_kernel_spmd(nc, in_maps, core_ids=list(range(NC)))
    if res.exec_time_ns is not None:
        print(f"HW exec time: {res.exec_time_ns} ns")
    out = np.zeros((B, S, D), np.float32)
    for c in range(NC):
        o = res.results[c]["out"]
        out[0, c::NC] = o[:CH]
        out[1, c::NC] = o[CH:]
    return out
